# revision 26
# baseline (speedup 1.0000x reference)
"""Trainium2 Bass kernel for a 2-layer GraphConv GNN + mean-pool + linear.

Reference computation (all fp32):
    h1 = leaky_relu(segsum(w*x[src] -> dst) @ W1_rel + x @ W1_root + b1)
    h2 = leaky_relu(segsum(w*h1[src] -> dst) @ W2_rel + h1 @ W2_root + b2)
    pooled = segment_mean(h2, batch, 512)
    out = pooled @ Wl_root + bl            # [512, 8]

Distribution (8 NeuronCores): nodes in contiguous ranges of 12500 per core;
edges on the dst-owning core; h1 exchanged with an AllGather split into 4
contiguous chunks (chunk-major h1_full layout) so the exchange overlaps the
tail of layer 1; per-graph pooling via one-hot matmul; the trivial
overlap-add + mean + final 64x8 linear run on host.

Design facts (measured on HW, not from the cost model):
  - dma_gather calls are limited to 1024 indices (2048 wedges the device);
    4 SWDGE queues in parallel sustain ~1.8 ns/idx. That descriptor
    generation (~600 us/core for 2x156k edges) is the critical resource.
  - Any concurrent DVE activity roughly halves gather throughput (shared
    SBUF ports), so the kernel does NOT build scatter one-hots on DVE.
    Instead dense per-chunk one-hot matrices (onehot[e,s] = w_e if
    s == dst_in_block[e]) are precomputed on the host as fp8_e4m3 and
    DMA-streamed from HBM (one DMA per super-block x residue span), and
    TensorE contracts them with the gathered bf16 rows into feature-major
    PSUM tiles.
  - Tails are software-pipelined in two lagged stages (PSUM->SBUF copy +
    z-matmuls + Lrelu, then transpose + export) and kept off the DVE queue
    so no engine queue head ever blocks cross-engine.

Per 128-edge chunk, per dst block b: agg[f,s] += sum_e g[e,f]*oh[e,s];
z = W_rel.T @ agg + W_root.T @ x_fm (two accumulating matmuls);
h = Lrelu(z + b) on ACT (alpha=0.01).

dma_gather constraints and how they're met:
  - elem stride %256B == 0 -> gather through 4 strided table views
    (elem_step = 4 rows of 128B bf16); idx = src//4 with edges grouped by
    residue r = src%4 (NPC=12500, NPAD=12544, and the chunk-major h1_full
    strides are all %4 == 0, so residues survive both table layouts).
  - int16 indices: row//4 < 25100 < 32768.
  - indices wrapped [i%16, i//16] into 16 partitions, replicated 8x down.
  - edge chunks laid out (super-block of 7 dst blocks, residue, block) so
    every call is a packed 8-chunk window over one residue table view,
    while consumption stays local to one super-block.
"""

import math

import numpy as np

import concourse.bacc as bacc
import concourse.bass as bass
import concourse.mybir as mybir
import concourse.tile as tile
from concourse.bass_utils import run_bass_kernel_spmd

F32 = mybir.dt.float32
FP8 = mybir.dt.float8e4
BF16 = mybir.dt.bfloat16
I16 = mybir.dt.int16
ALU = mybir.AluOpType
ACTF = mybir.ActivationFunctionType

NRES = 4       # residue groups (table views); stride = 4 rows = 512B
CALL_CHUNKS = 8   # chunks per dma_gather call (1024 idx = HW limit)
NQ = 4         # SWDGE queues (Q7 core pairs) to rotate gather calls over
KCH = 8        # AllGather chunks (overlap the exchange with layer-1 tails)
USE_LRELU = True  # single ACT Lrelu op (alpha param); fallback: mult+max


class Cfg:
    def __init__(self, n_nodes, n_edges, d, n_graphs, n_cores=8, block=128):
        assert n_nodes % n_cores == 0
        self.N = n_nodes
        self.E = n_edges
        self.D = d
        self.G = n_graphs
        self.CORES = n_cores
        self.BLOCK = block
        self.NPC = n_nodes // n_cores                    # nodes per core
        self.NB = math.ceil(self.NPC / block)            # blocks per core
        self.NPAD = self.NB * block                      # padded nodes/core
        self.LEAKY = 0.01
        assert self.NPC % NRES == 0 and self.NPAD % NRES == 0


REAL_CFG = Cfg(100000, 1250000, 64, 512)


def _dma_gather_any(nc, out_ap, in_ap, idxs_ap, num_idxs, elem_size,
                    elem_step, queue_num):
    """dma_gather with the 256B-element restriction relaxed to 128B.

    Mirrors concourse.bass.BassGpSimd.dma_gather (non-transpose, HBM source);
    the ucode (dma_gather.cpp) computes descriptor lengths from
    elem_size*dtype_size generically -- only stride_bytes must divide by 256.
    """
    import concourse.ap_utils as ap_utils
    gp = nc.gpsimd
    gp._assert_queue_num(queue_num)
    assert idxs_ap.dtype == mybir.dt.int16
    assert in_ap.dtype == out_ap.dtype
    elem_size_bytes = elem_size * mybir.dt.size(in_ap.dtype)
    assert elem_size_bytes % 128 == 0
    assert ap_utils.ap_is_contiguous(in_ap.ap[1:])
    assert ap_utils.ap_is_contiguous(out_ap.ap[1:])
    assert ap_utils.ap_is_contiguous(idxs_ap.ap[1:])
    assert in_ap.ap[-1][1] == out_ap.ap[-1][1] == elem_size
    assert out_ap.ap[0][1] * out_ap.ap[1][1] == ((num_idxs + 127) // 128) * 128
    assert in_ap.ap[0][0] == elem_step
    stride_bytes = elem_step * mybir.dt.size(in_ap.dtype)
    assert stride_bytes % 256 == 0
    _in_ap = gp.lower_ap_dma(in_ap, for_custom_bir_dma=True)
    _idxs_ap = gp.lower_ap(idxs_ap)
    _out_ap = gp.lower_ap(out_ap)
    return gp.add_instruction(
        mybir.InstDMAGatherAnt(
            name=nc.get_next_instruction_name(),
            ins=[*_in_ap, _idxs_ap,
                 gp.lower_val_access(gp.to_reg(num_idxs))],
            outs=[_out_ap],
            transpose=False,
            num_idxs=num_idxs,
            elem_size=elem_size,
            stride_bytes_256=stride_bytes // 256,
            gen_mode=0,
            single_packet=True,
            queue_num=queue_num,
            sbuf_tokens_per_rank=0,
            sbuf_free_dim_per_rank=0,
            sbuf_free_dim_pad_per_rank=0,
            sbuf_byte_offset=0,
        ))


# ---------------------------------------------------------------------------
# Host-side preprocessing: shard edges by dst core; group per (src%4,
# dst-half-block) with residue OUTERMOST so chunks of one residue are
# contiguous (gather calls are then fully packed 8-chunk windows over one
# strided table view); pad each group to chunks of 128; build int16 index
# streams wrapped for dma_gather.
# ---------------------------------------------------------------------------

def _wrap16(idx):
    """[n] int -> [128, n//16] int16: i at [i%16, i//16], replicated 8x."""
    n = len(idx)
    w = np.ascontiguousarray(idx.reshape(n // 16, 16).T).astype(np.int16)
    return np.tile(w, (8, 1))


def preprocess(cfg, x, edge_index, weights, batch):
    N, E, D, CORES = cfg.N, cfg.E, cfg.D, cfg.CORES
    NPC, NB, NPAD, BLOCK = cfg.NPC, cfg.NB, cfg.NPAD, cfg.BLOCK

    src = np.asarray(edge_index[0], dtype=np.int64)
    dst = np.asarray(edge_index[1], dtype=np.int64)
    w = np.asarray(weights, dtype=np.float32)
    batch = np.asarray(batch, dtype=np.int64)

    # super-blocks of SBW dst blocks; columns ordered (sb, residue, block)
    # so gather calls are packed windows over one residue table view while
    # consumption (dst-block order) stays local to one super-block.
    SBW = 1
    for cand in (7, 14, 16, 8, 4, 2):
        if NB % cand == 0:
            SBW = cand
            break
    NSB = NB // SBW

    core_of = dst // NPC
    ld = dst - core_of * NPC
    blk = ld // BLOCK
    dib = ld - blk * BLOCK
    res = src % NRES
    sb = blk // SBW
    b_in = blk - sb * SBW
    order = np.lexsort((b_in, res, sb, core_of))
    src_s, w_s = src[order], w[order]
    core_s, dib_s = core_of[order], dib[order].astype(np.float32)

    gl = ((sb * NRES + res) * SBW + b_in)[order]         # group within core
    gid = core_s * (NRES * NB) + gl
    NG = CORES * NRES * NB
    counts = np.bincount(gid, minlength=NG).reshape(CORES, NRES * NB)
    # chunks per group: max over cores (SPMD identical program)
    K = np.maximum(1, -(-counts.max(axis=0) // BLOCK))   # [NRES*NB]
    cs = np.concatenate([[0], np.cumsum(K)])             # chunk col offsets
    C = int(cs[-1])                                      # chunks per core

    first = np.concatenate([[0], np.cumsum(counts.reshape(-1))])[:-1]
    rank = np.arange(E, dtype=np.int64) - first[gid]
    slot = cs[gl] * BLOCK + rank                         # slot within core

    q1 = np.zeros((CORES, C * BLOCK), dtype=np.int16)
    q2 = np.zeros((CORES, C * BLOCK), dtype=np.int16)
    dibp = np.full((CORES, C * BLOCK), -1.0, dtype=np.float32)
    wp = np.zeros((CORES, C * BLOCK), dtype=np.float32)
    q1[core_s, slot] = (src_s // NRES).astype(np.int16)
    # h1_full is laid out chunk-major: [KCH, CORES, NPAD//KCH, 64] so each
    # chunked AllGather writes a contiguous slab. All strides are %4 == 0,
    # so src%4 residues survive.
    rows_k = NPAD // KCH
    l_loc = src_s % NPC
    k_ch = l_loc // rows_k
    sp = (k_ch * CORES + (src_s // NPC)) * rows_k + (l_loc - k_ch * rows_k)
    q2[core_s, slot] = (sp // NRES).astype(np.int16)
    dibp[core_s, slot] = dib_s
    wp[core_s, slot] = w_s

    # gather calls: per (sb, residue), packed CALL_CHUNKS windows; the
    # prebuilt one-hot stream is DMA'd per (sb, residue) span
    calls = []                                           # (r, c0, nchunks)
    ohspans = []                                         # (c0, c1)
    for s in range(NSB):
        for r in range(NRES):
            g0 = (s * NRES + r) * SBW
            r0, r1 = int(cs[g0]), int(cs[g0 + SBW])
            ohspans.append((r0, r1))
            c0 = r0
            while c0 < r1:
                kk = min(CALL_CHUNKS, r1 - c0)
                calls.append((r, c0, kk))
                c0 += kk

    # per-block chunk columns for the matmul bookkeeping
    blk_chunks = [[] for _ in range(NB)]                 # block -> [cols]
    for b in range(NB):
        s, bi = b // SBW, b % SBW
        for r in range(NRES):
            g = (s * NRES + r) * SBW + bi
            blk_chunks[b].extend(range(int(cs[g]), int(cs[g + 1])))

    import jax.numpy as jnp
    import ml_dtypes
    x_bf16 = np.asarray(jnp.asarray(x, dtype=jnp.bfloat16))

    # dense one-hot stream: ohd[p, cc*128 + s] = w * (s == dib) per edge
    # (slot p of chunk cc); identical for both layers.
    def build_ohd(dibp_c, wp_c):
        ohd = np.zeros(128 * C * BLOCK, dtype=ml_dtypes.float8_e4m3fn)
        sl = np.nonzero(dibp_c >= 0)[0]
        p_a, cc_a = sl % BLOCK, sl // BLOCK
        flat = p_a * (C * BLOCK) + cc_a * BLOCK + dibp_c[sl].astype(np.int64)
        ohd[flat] = wp_c[sl]
        return ohd.reshape(128, C * BLOCK)

    g_base = batch[np.arange(CORES) * NPC]
    in_maps = []
    for c in range(CORES):
        xs = x_bf16[c * NPC:(c + 1) * NPC]
        xT = np.zeros((D, NPAD), dtype=x_bf16.dtype)
        xT[:, :NPC] = xs.T
        gs = np.full(NPAD, -1.0, dtype=np.float32)
        gs[:NPC] = (batch[c * NPC:(c + 1) * NPC] - g_base[c]).astype(
            np.float32)
        assert gs.max() < 128.0, "graph span per core exceeds 128"
        in_maps.append({
            "x": np.ascontiguousarray(x_bf16),
            "xT": np.ascontiguousarray(xT),
            "idx1": _wrap16(q1[c]),
            "idx2": _wrap16(q2[c]),
            "ohd": build_ohd(dibp[c], wp[c]),
            "gslot": np.ascontiguousarray(gs.reshape(NB, BLOCK).T),
        })
    plan = {"calls": calls, "blk_chunks": blk_chunks, "C": C,
            "ohspans": ohspans}
    return in_maps, plan, g_base


# ---------------------------------------------------------------------------
# Bass program
# ---------------------------------------------------------------------------

def build_nc(cfg, plan, reps=1, ablate=()):
    """plan: from preprocess (gather call windows + per-block chunk cols).
    reps>1 repeats the computation (timing: the delta between an n-rep and
    1-rep program cancels the axon dispatch floor). ablate: subset of
    {"nogather","noonehot","noagg","nocoll"} for timing A/B experiments."""
    ablate = frozenset(ablate)
    N, D, CORES = cfg.N, cfg.D, cfg.CORES
    NB, NPAD = cfg.NB, cfg.NPAD
    C = plan["C"]
    calls = plan["calls"]
    blk_chunks = plan["blk_chunks"]
    ohspans = plan["ohspans"]

    nc = bacc.Bacc("TRN2", target_bir_lowering=False, debug=False,
                   num_devices=CORES, num_swdge_queues=NQ)

    x_d = nc.dram_tensor("x", [N, D], BF16, kind="ExternalInput")
    xT_d = nc.dram_tensor("xT", [D, NPAD], BF16, kind="ExternalInput")
    idx1_d = nc.dram_tensor("idx1", [128, C * 8], I16, kind="ExternalInput")
    idx2_d = nc.dram_tensor("idx2", [128, C * 8], I16, kind="ExternalInput")
    ohd_d = nc.dram_tensor("ohd", [128, C * 128], FP8,
                           kind="ExternalInput")
    gslot_d = nc.dram_tensor("gslot", [128, NB], F32, kind="ExternalInput")
    w1c_d = nc.dram_tensor("W1c", [D, 2 * D], BF16, kind="ExternalInput")
    w2c_d = nc.dram_tensor("W2c", [D, 2 * D], BF16, kind="ExternalInput")
    b1_d = nc.dram_tensor("b1", [D, 1], F32, kind="ExternalInput")
    b2_d = nc.dram_tensor("b2", [D, 1], F32, kind="ExternalInput")
    iota_d = nc.dram_tensor("iota", [128, 128], BF16, kind="ExternalInput")
    id64_d = nc.dram_tensor("id64", [D, D], BF16, kind="ExternalInput")

    pool_d = nc.dram_tensor("pool", [128, D], F32, kind="ExternalOutput")

    h1_local = nc.dram_tensor("h1_local", [NPAD, D], BF16)
    h1_full = nc.dram_tensor("h1_full", [NPAD * CORES, D], BF16,
                             addr_space="Shared")

    with tile.TileContext(nc) as tc:
        with (
            tc.tile_pool(name="persist", bufs=1) as pp,
            tc.tile_pool(name="work", bufs=6) as wp,
            tc.tile_pool(name="h2p", bufs=8) as h2p,
            tc.tile_pool(name="gat", bufs=40) as gp,
            tc.tile_pool(name="ohp", bufs=9) as ohp,
            tc.tile_pool(name="agg", bufs=3, space="PSUM") as aggp,
            tc.tile_pool(name="ps", bufs=2, space="PSUM") as psp,
            tc.tile_pool(name="pool1", bufs=1, space="PSUM") as pool1,
        ):
            xT_s = pp.tile([D, NPAD], BF16, tag="xT")
            h1T_s = pp.tile([D, NPAD], BF16, tag="h1T")
            idx1_s = pp.tile([128, C * 8], I16, tag="idx1")
            idx2_s = pp.tile([128, C * 8], I16, tag="idx2")
            gslot_s = pp.tile([128, NB], F32, tag="gslot")
            w1c_s = pp.tile([D, 2 * D], BF16, tag="w1c")
            w2c_s = pp.tile([D, 2 * D], BF16, tag="w2c")
            b1_s = pp.tile([D, 1], F32, tag="b1")
            b2_s = pp.tile([D, 1], F32, tag="b2")
            iota_s = pp.tile([128, 128], BF16, tag="iota")
            id64_s = pp.tile([D, D], BF16, tag="id64")

            for t, d in [(xT_s, xT_d), (idx1_s, idx1_d), (idx2_s, idx2_d),
                         (gslot_s, gslot_d),
                         (w1c_s, w1c_d), (w2c_s, w2c_d), (b1_s, b1_d),
                         (b2_s, b2_d), (iota_s, iota_d), (id64_s, id64_d)]:
                nc.sync.dma_start(out=t[:], in_=d[:, :])

            pool_ps = pool1.tile([128, D], F32, tag="pool")
            qctr = [0]

            if ablate:
                gdummy = pp.tile([128, CALL_CHUNKS * 64], BF16, tag="gdum")
                for k8 in range(CALL_CHUNKS):
                    nc.sync.dma_start(out=gdummy[:, k8 * 64:(k8 + 1) * 64],
                                      in_=x_d[0:128, :])
                ohdummy = pp.tile([128, 128], FP8, tag="ohdum")
                nc.sync.dma_start(out=ohdummy[:], in_=ohd_d[:, 0:128])

            LAG = 2  # blocks of tail pipelining (PSUM agg bufs = LAG+1)

            def leaky(dst_ap, z_ps, b_s):
                if USE_LRELU:
                    nc.scalar.activation(out=dst_ap, in_=z_ps[:],
                                         func=ACTF.Lrelu, bias=b_s[:, 0:1],
                                         alpha=cfg.LEAKY)
                else:
                    zb = wp.tile([D, 128], BF16, tag="zb")
                    nc.scalar.activation(out=zb[:], in_=z_ps[:],
                                         func=ACTF.Identity,
                                         bias=b_s[:, 0:1])
                    t = wp.tile([D, 128], BF16, tag="zt")
                    nc.vector.tensor_scalar_mul(out=t[:], in0=zb[:],
                                                scalar1=cfg.LEAKY)
                    nc.vector.tensor_tensor(out=dst_ap, in0=zb[:], in1=t[:],
                                            op=ALU.max)

            def layer(idx_s, table_d, wc_s, b_s, xfm_s, tail1_dst,
                      tail2_fn):
                # strided views: row stride 4 rows (512B), base offset r rows
                tabv = table_d[:, :].rearrange("(a b) f -> a (b f)", b=NRES)
                gtiles = []          # (c0, kk, tile) in call order
                pending1 = []        # blocks awaiting stage-1 tail
                pending2 = []        # blocks awaiting stage-2 tail

                def tail1(b, agg_ps):
                    # stage 1, LAG1 blocks late: PSUM->SBUF copy (ACT), the
                    # two accumulating z matmuls (PE), leaky (ACT). By now
                    # the agg accumulation finished LAG1 blocks ago, so the
                    # ACT copy at the queue head never waits on PE.
                    agg_sb = wp.tile([D, 128], BF16, tag="aggsb")
                    nc.scalar.activation(out=agg_sb[:], in_=agg_ps[:],
                                         func=ACTF.Copy)
                    z_ps = psp.tile([D, 128], F32, tag="z")
                    nc.tensor.matmul(out=z_ps[:], lhsT=wc_s[:, 0:D],
                                     rhs=agg_sb[:], start=True, stop=False)
                    nc.tensor.matmul(out=z_ps[:], lhsT=wc_s[:, D:2 * D],
                                     rhs=xfm_s[:, b * 128:(b + 1) * 128],
                                     start=False, stop=True)
                    leaky(tail1_dst(b), z_ps, b_s)
                    pending2.append(b)

                def flush_calls(upto_chunk):
                    # issue gather calls covering chunks < upto_chunk
                    while len(gtiles) < len(calls):
                        r, c0, kk = calls[len(gtiles)]
                        if c0 >= upto_chunk:
                            break
                        if "nogather" in ablate:
                            gtiles.append((c0, kk, gdummy))
                            continue
                        g = gp.tile([128, kk * 64], BF16, tag="g")
                        gv = g[:].rearrange("p (c f) -> p c f", c=kk)
                        _dma_gather_any(
                            nc, gv, tabv[:, r * 64:(r + 1) * 64],
                            idx_s[:, c0 * 8:(c0 + kk) * 8],
                            kk * 128, 64, NRES * 64, qctr[0] % NQ)
                        qctr[0] += 1
                        gtiles.append((c0, kk, g))

                call_of = {}
                for ci, (r, c0, kk) in enumerate(calls):
                    for cc in range(c0, c0 + kk):
                        call_of[cc] = ci

                ohtiles = []         # (c0, c1, tile) per (sb, r) span

                def flush_oh(upto_chunk):
                    while len(ohtiles) < len(ohspans):
                        c0, c1 = ohspans[len(ohtiles)]
                        if c0 >= upto_chunk:
                            break
                        if "noonehot" in ablate:
                            ohtiles.append((c0, c1, None))
                            continue
                        t = ohp.tile([128, (c1 - c0) * 128], FP8, tag="oh")
                        nc.sync.dma_start(
                            out=t[:], in_=ohd_d[:, c0 * 128:c1 * 128])
                        ohtiles.append((c0, c1, t))

                span_of = {}
                for si, (c0, c1) in enumerate(ohspans):
                    for cc in range(c0, c1):
                        span_of[cc] = si

                for b in range(NB):
                    cols = blk_chunks[b]
                    need = max(cols) + 1 if cols else 0
                    flush_calls(need)
                    flush_oh(need + 25)
                    agg_ps = aggp.tile([D, 128], F32, tag="agg")
                    total = len(cols)
                    work = []
                    for cc in cols:
                        ci = call_of[cc]
                        c0, kk, g = gtiles[ci]
                        col = cc - c0
                        o0, o1, ot = ohtiles[span_of[cc]]
                        if ot is None:
                            work.append((g, col, ohdummy, 0))
                        else:
                            work.append((g, col, ot, cc - o0))
                    if "noagg" in ablate:
                        nc.tensor.matmul(
                            out=agg_ps[:], lhsT=gdummy[:, 0:64],
                            rhs=ohdummy[:], start=True, stop=True)
                        work = []
                    done = 0
                    for (g, col, ot, ocol) in work:
                        nc.tensor.matmul(
                            out=agg_ps[:],
                            lhsT=g[:, col * 64:(col + 1) * 64],
                            rhs=ot[:, ocol * 128:(ocol + 1) * 128],
                            start=(done == 0),
                            stop=(done == total - 1))
                        done += 1
                    pending1.append((b, agg_ps))
                    if len(pending1) > LAG:
                        tail1(*pending1.pop(0))
                    if len(pending2) > LAG:
                        tail2_fn(pending2.pop(0))
                while pending1:
                    tail1(*pending1.pop(0))
                while pending2:
                    tail2_fn(pending2.pop(0))

            def l1_dst(b):
                return h1T_s[:, b * 128:(b + 1) * 128]

            def l1_tail2(b):
                # stage 2, 2*LAG blocks late: by now leaky(b) is long done,
                # so PE never stalls at the transpose. h1 rows go to DRAM
                # straight from PSUM (no compute-engine copy).
                t_ps = psp.tile([128, D], BF16, tag="tp")
                nc.tensor.transpose(out=t_ps[:],
                                    in_=h1T_s[:, b * 128:(b + 1) * 128],
                                    identity=id64_s[:])
                h1nm = wp.tile([128, D], BF16, tag="h1nm")
                nc.scalar.activation(out=h1nm[:], in_=t_ps[:],
                                     func=ACTF.Copy)
                nc.sync.dma_start(out=h1_local[b * 128:(b + 1) * 128, :],
                                  in_=h1nm[:])

            for _rep in range(reps):
                layer(idx1_s, x_d, w1c_s, b1_s, xT_s, l1_dst, l1_tail2)

                if "nocoll" not in ablate:
                    rows_k = NPAD // KCH
                    for k in range(KCH):
                        nc.gpsimd.collective_compute(
                            "AllGather",
                            ALU.bypass,
                            replica_groups=[list(range(CORES))],
                            ins=[h1_local[k * rows_k:(k + 1) * rows_k, :]],
                            outs=[h1_full[k * CORES * rows_k:
                                          (k + 1) * CORES * rows_k, :]],
                        )

                h2fm_tiles = {}

                def l2_dst(b):
                    t = h2p.tile([D, 128], BF16, tag="h2fm")
                    h2fm_tiles[b] = t
                    return t[:]

                def l2_tail2(b):
                    t_ps = psp.tile([128, D], BF16, tag="tp")
                    nc.tensor.transpose(out=t_ps[:],
                                        in_=h2fm_tiles.pop(b)[:],
                                        identity=id64_s[:])
                    h2nm = wp.tile([128, D], BF16, tag="h2nm")
                    nc.scalar.activation(out=h2nm[:], in_=t_ps[:],
                                         func=ACTF.Copy)
                    ph = wp.tile([128, 128], BF16, tag="ph")
                    nc.vector.tensor_scalar(
                        out=ph[:], in0=iota_s[:],
                        scalar1=gslot_s[:, b:b + 1], scalar2=None,
                        op0=ALU.is_equal)
                    nc.tensor.matmul(out=pool_ps[:], lhsT=ph[:],
                                     rhs=h2nm[:],
                                     start=(b == 0), stop=(b == NB - 1))

                layer(idx2_s, h1_full, w2c_s, b2_s, h1T_s, l2_dst, l2_tail2)

            pool_s = wp.tile([128, D], F32, tag="pools")
            nc.scalar.activation(out=pool_s[:], in_=pool_ps[:],
                                 func=ACTF.Copy)
            nc.sync.dma_start(out=pool_d[:, :], in_=pool_s[:])

    nc.compile()
    return nc


# ---------------------------------------------------------------------------
# Entry point
# ---------------------------------------------------------------------------

_CACHE = {}


def _common_inputs(cfg, W1_root, W1_rel, W2_root, W2_rel, b1, b2):
    D = cfg.D
    import jax.numpy as jnp

    def bf(a):
        return np.asarray(jnp.asarray(np.asarray(a, np.float32),
                                      dtype=jnp.bfloat16))

    return {
        "W1c": bf(np.concatenate([W1_rel, W1_root], axis=1)),
        "W2c": bf(np.concatenate([W2_rel, W2_root], axis=1)),
        "b1": np.ascontiguousarray(
            np.asarray(b1, np.float32).reshape(D, 1)),
        "b2": np.ascontiguousarray(
            np.asarray(b2, np.float32).reshape(D, 1)),
        "iota": bf(np.broadcast_to(np.arange(128, dtype=np.float32),
                                   (128, 128)).copy()),
        "id64": bf(np.eye(D, dtype=np.float32)),
    }


def _plan_key(plan):
    return (tuple(plan["calls"]), tuple(plan["ohspans"]),
            tuple(tuple(c) for c in plan["blk_chunks"]), plan["C"])


def run(cfg, inputs, trace=False):
    x = np.asarray(inputs["x_embeddings"], dtype=np.float32)
    in_maps, plan, g_base = preprocess(
        cfg, x, inputs["edge_index"], inputs["weights"], inputs["batch"])
    common = _common_inputs(cfg, inputs["W1_root"], inputs["W1_rel"],
                            inputs["W2_root"], inputs["W2_rel"],
                            inputs["b1"], inputs["b2"])
    for m in in_maps:
        m.update(common)

    key = (cfg.N, cfg.E, _plan_key(plan))
    if key not in _CACHE:
        _CACHE[key] = build_nc(cfg, plan)
    nc = _CACHE[key]

    res = run_bass_kernel_spmd(nc, in_maps, core_ids=list(range(cfg.CORES)),
                               trace=trace)

    batch = np.asarray(inputs["batch"], dtype=np.int64)
    counts = np.bincount(batch, minlength=cfg.G).astype(np.float32)
    pooled = np.zeros((cfg.G + 128, cfg.D), dtype=np.float32)
    for c in range(cfg.CORES):
        pooled[g_base[c]:g_base[c] + 128] += res.results[c]["pool"]
    pooled = pooled[:cfg.G] / np.maximum(counts, 1.0)[:, None]
    out = pooled @ np.asarray(inputs["Wl_root"], dtype=np.float32)
    out = out + np.asarray(inputs["bl"], dtype=np.float32)
    return out.astype(np.float32), res


def kernel(**inputs) -> np.ndarray:
    out, _ = run(REAL_CFG, inputs, trace=False)
    return out


# revision 27
# speedup vs baseline: 1.1334x; 1.1334x over previous
"""Trainium2 Bass kernel for a 2-layer GraphConv GNN + mean-pool + linear.

Reference computation (all fp32):
    h1 = leaky_relu(segsum(w*x[src] -> dst) @ W1_rel + x @ W1_root + b1)
    h2 = leaky_relu(segsum(w*h1[src] -> dst) @ W2_rel + h1 @ W2_root + b2)
    pooled = segment_mean(h2, batch, 512)
    out = pooled @ Wl_root + bl            # [512, 8]

Distribution (8 NeuronCores): nodes in contiguous ranges of 12500 per core;
edges on the dst-owning core; h1 exchanged with an AllGather split into 4
contiguous chunks (chunk-major h1_full layout) so the exchange overlaps the
tail of layer 1; per-graph pooling via one-hot matmul; the trivial
overlap-add + mean + final 64x8 linear run on host.

Design facts (measured on HW, not from the cost model):
  - dma_gather calls are limited to 1024 indices (2048 wedges the device);
    4 SWDGE queues in parallel sustain ~1.8 ns/idx. That descriptor
    generation (~600 us/core for 2x156k edges) is the critical resource.
  - Any concurrent DVE activity roughly halves gather throughput (shared
    SBUF ports), so the kernel does NOT build scatter one-hots on DVE.
    Instead dense per-chunk one-hot matrices (onehot[e,s] = w_e if
    s == dst_in_block[e]) are precomputed on the host as fp8_e4m3 and
    DMA-streamed from HBM (one DMA per super-block x residue span), and
    TensorE contracts them with the gathered bf16 rows into feature-major
    PSUM tiles.
  - Tails are software-pipelined in two lagged stages (PSUM->SBUF copy +
    z-matmuls + Lrelu, then transpose + export) and kept off the DVE queue
    so no engine queue head ever blocks cross-engine.

Per 128-edge chunk, per dst block b: agg[f,s] += sum_e g[e,f]*oh[e,s];
z = W_rel.T @ agg + W_root.T @ x_fm (two accumulating matmuls);
h = Lrelu(z + b) on ACT (alpha=0.01).

dma_gather constraints and how they're met:
  - elem stride %256B == 0 -> gather through 4 strided table views
    (elem_step = 4 rows of 128B bf16); idx = src//4 with edges grouped by
    residue r = src%4 (NPC=12500, NPAD=12544, and the chunk-major h1_full
    strides are all %4 == 0, so residues survive both table layouts).
  - int16 indices: row//4 < 25100 < 32768.
  - indices wrapped [i%16, i//16] into 16 partitions, replicated 8x down.
  - edge chunks laid out (super-block of 7 dst blocks, residue, block) so
    every call is a packed 8-chunk window over one residue table view,
    while consumption stays local to one super-block.
"""

import math

import numpy as np

import concourse.bacc as bacc
import concourse.bass as bass
import concourse.mybir as mybir
import concourse.tile as tile
from concourse.bass_utils import run_bass_kernel_spmd

F32 = mybir.dt.float32
FP8 = mybir.dt.float8e4
BF16 = mybir.dt.bfloat16
I16 = mybir.dt.int16
ALU = mybir.AluOpType
ACTF = mybir.ActivationFunctionType

NRES = 4       # residue groups (table views); stride = 4 rows = 512B
CALL_CHUNKS = 8   # chunks per dma_gather call (1024 idx = HW limit)
NQ = 4         # SWDGE queues (Q7 core pairs) to rotate gather calls over
KCH = 4        # AllGather chunks (overlap the exchange with layer-1 tails)
USE_LRELU = True  # single ACT Lrelu op (alpha param); fallback: mult+max


class Cfg:
    def __init__(self, n_nodes, n_edges, d, n_graphs, n_cores=8, block=128):
        assert n_nodes % n_cores == 0
        self.N = n_nodes
        self.E = n_edges
        self.D = d
        self.G = n_graphs
        self.CORES = n_cores
        self.BLOCK = block
        self.NPC = n_nodes // n_cores                    # nodes per core
        self.NB = math.ceil(self.NPC / block)            # blocks per core
        self.NPAD = self.NB * block                      # padded nodes/core
        self.LEAKY = 0.01
        assert self.NPC % NRES == 0 and self.NPAD % NRES == 0


REAL_CFG = Cfg(100000, 1250000, 64, 512)


def _dma_gather_any(nc, out_ap, in_ap, idxs_ap, num_idxs, elem_size,
                    elem_step, queue_num):
    """dma_gather with the 256B-element restriction relaxed to 128B.

    Mirrors concourse.bass.BassGpSimd.dma_gather (non-transpose, HBM source);
    the ucode (dma_gather.cpp) computes descriptor lengths from
    elem_size*dtype_size generically -- only stride_bytes must divide by 256.
    """
    import concourse.ap_utils as ap_utils
    gp = nc.gpsimd
    gp._assert_queue_num(queue_num)
    assert idxs_ap.dtype == mybir.dt.int16
    assert in_ap.dtype == out_ap.dtype
    elem_size_bytes = elem_size * mybir.dt.size(in_ap.dtype)
    assert elem_size_bytes % 128 == 0
    assert ap_utils.ap_is_contiguous(in_ap.ap[1:])
    assert ap_utils.ap_is_contiguous(out_ap.ap[1:])
    assert ap_utils.ap_is_contiguous(idxs_ap.ap[1:])
    assert in_ap.ap[-1][1] == out_ap.ap[-1][1] == elem_size
    assert out_ap.ap[0][1] * out_ap.ap[1][1] == ((num_idxs + 127) // 128) * 128
    assert in_ap.ap[0][0] == elem_step
    stride_bytes = elem_step * mybir.dt.size(in_ap.dtype)
    assert stride_bytes % 256 == 0
    _in_ap = gp.lower_ap_dma(in_ap, for_custom_bir_dma=True)
    _idxs_ap = gp.lower_ap(idxs_ap)
    _out_ap = gp.lower_ap(out_ap)
    return gp.add_instruction(
        mybir.InstDMAGatherAnt(
            name=nc.get_next_instruction_name(),
            ins=[*_in_ap, _idxs_ap,
                 gp.lower_val_access(gp.to_reg(num_idxs))],
            outs=[_out_ap],
            transpose=False,
            num_idxs=num_idxs,
            elem_size=elem_size,
            stride_bytes_256=stride_bytes // 256,
            gen_mode=0,
            single_packet=True,
            queue_num=queue_num,
            sbuf_tokens_per_rank=0,
            sbuf_free_dim_per_rank=0,
            sbuf_free_dim_pad_per_rank=0,
            sbuf_byte_offset=0,
        ))


# ---------------------------------------------------------------------------
# Host-side preprocessing: shard edges by dst core; group per (src%4,
# dst-half-block) with residue OUTERMOST so chunks of one residue are
# contiguous (gather calls are then fully packed 8-chunk windows over one
# strided table view); pad each group to chunks of 128; build int16 index
# streams wrapped for dma_gather.
# ---------------------------------------------------------------------------

def _wrap16(idx):
    """[n] int -> [128, n//16] int16: i at [i%16, i//16], replicated 8x."""
    n = len(idx)
    w = np.ascontiguousarray(idx.reshape(n // 16, 16).T).astype(np.int16)
    return np.tile(w, (8, 1))


def preprocess(cfg, x, edge_index, weights, batch):
    N, E, D, CORES = cfg.N, cfg.E, cfg.D, cfg.CORES
    NPC, NB, NPAD, BLOCK = cfg.NPC, cfg.NB, cfg.NPAD, cfg.BLOCK

    src = np.asarray(edge_index[0], dtype=np.int64)
    dst = np.asarray(edge_index[1], dtype=np.int64)
    w = np.asarray(weights, dtype=np.float32)
    batch = np.asarray(batch, dtype=np.int64)

    # super-blocks of SBW dst blocks; columns ordered (sb, residue, block)
    # so gather calls are packed windows over one residue table view while
    # consumption (dst-block order) stays local to one super-block.
    SBW = 1
    for cand in (7, 14, 16, 8, 4, 2):
        if NB % cand == 0:
            SBW = cand
            break
    NSB = NB // SBW

    core_of = dst // NPC
    ld = dst - core_of * NPC
    blk = ld // BLOCK
    dib = ld - blk * BLOCK
    res = src % NRES
    sb = blk // SBW
    b_in = blk - sb * SBW
    order = np.lexsort((b_in, res, sb, core_of))
    src_s, w_s = src[order], w[order]
    core_s, dib_s = core_of[order], dib[order].astype(np.float32)

    gl = ((sb * NRES + res) * SBW + b_in)[order]         # group within core
    gid = core_s * (NRES * NB) + gl
    NG = CORES * NRES * NB
    counts = np.bincount(gid, minlength=NG).reshape(CORES, NRES * NB)
    # chunks per group: max over cores (SPMD identical program)
    K = np.maximum(1, -(-counts.max(axis=0) // BLOCK))   # [NRES*NB]
    cs = np.concatenate([[0], np.cumsum(K)])             # chunk col offsets
    C = int(cs[-1])                                      # chunks per core

    first = np.concatenate([[0], np.cumsum(counts.reshape(-1))])[:-1]
    rank = np.arange(E, dtype=np.int64) - first[gid]
    slot = cs[gl] * BLOCK + rank                         # slot within core

    q1 = np.zeros((CORES, C * BLOCK), dtype=np.int16)
    q2 = np.zeros((CORES, C * BLOCK), dtype=np.int16)
    dibp = np.full((CORES, C * BLOCK), -1.0, dtype=np.float32)
    wp = np.zeros((CORES, C * BLOCK), dtype=np.float32)
    q1[core_s, slot] = (src_s // NRES).astype(np.int16)
    # h1_full is laid out chunk-major: [KCH, CORES, NPAD//KCH, 64] so each
    # chunked AllGather writes a contiguous slab. All strides are %4 == 0,
    # so src%4 residues survive.
    rows_k = NPAD // KCH
    l_loc = src_s % NPC
    k_ch = l_loc // rows_k
    sp = (k_ch * CORES + (src_s // NPC)) * rows_k + (l_loc - k_ch * rows_k)
    q2[core_s, slot] = (sp // NRES).astype(np.int16)
    dibp[core_s, slot] = dib_s
    wp[core_s, slot] = w_s

    # gather calls: per (sb, residue), packed CALL_CHUNKS windows; the
    # prebuilt one-hot stream is DMA'd per (sb, residue) span
    calls = []                                           # (r, c0, nchunks)
    ohspans = []                                         # (c0, c1)
    for s in range(NSB):
        for r in range(NRES):
            g0 = (s * NRES + r) * SBW
            r0, r1 = int(cs[g0]), int(cs[g0 + SBW])
            ohspans.append((r0, r1))
            c0 = r0
            while c0 < r1:
                kk = min(CALL_CHUNKS, r1 - c0)
                calls.append((r, c0, kk))
                c0 += kk

    # per-block chunk columns for the matmul bookkeeping
    blk_chunks = [[] for _ in range(NB)]                 # block -> [cols]
    for b in range(NB):
        s, bi = b // SBW, b % SBW
        for r in range(NRES):
            g = (s * NRES + r) * SBW + bi
            blk_chunks[b].extend(range(int(cs[g]), int(cs[g + 1])))

    import jax.numpy as jnp
    import ml_dtypes
    x_bf16 = np.asarray(jnp.asarray(x, dtype=jnp.bfloat16))

    # dense one-hot stream: ohd[p, cc*128 + s] = w * (s == dib) per edge
    # (slot p of chunk cc); identical for both layers.
    def build_ohd(dibp_c, wp_c):
        ohd = np.zeros(128 * C * BLOCK, dtype=ml_dtypes.float8_e4m3fn)
        sl = np.nonzero(dibp_c >= 0)[0]
        p_a, cc_a = sl % BLOCK, sl // BLOCK
        flat = p_a * (C * BLOCK) + cc_a * BLOCK + dibp_c[sl].astype(np.int64)
        ohd[flat] = wp_c[sl]
        return ohd.reshape(128, C * BLOCK)

    g_base = batch[np.arange(CORES) * NPC]
    in_maps = []
    for c in range(CORES):
        xs = x_bf16[c * NPC:(c + 1) * NPC]
        xT = np.zeros((D, NPAD), dtype=x_bf16.dtype)
        xT[:, :NPC] = xs.T
        gs = np.full(NPAD, -1.0, dtype=np.float32)
        gs[:NPC] = (batch[c * NPC:(c + 1) * NPC] - g_base[c]).astype(
            np.float32)
        assert gs.max() < 128.0, "graph span per core exceeds 128"
        in_maps.append({
            "x": np.ascontiguousarray(x_bf16),
            "xT": np.ascontiguousarray(xT),
            "idx1": _wrap16(q1[c]),
            "idx2": _wrap16(q2[c]),
            "ohd": build_ohd(dibp[c], wp[c]),
            "gslot": np.ascontiguousarray(gs.reshape(NB, BLOCK).T),
        })
    plan = {"calls": calls, "blk_chunks": blk_chunks, "C": C,
            "ohspans": ohspans}
    return in_maps, plan, g_base


# ---------------------------------------------------------------------------
# Bass program
# ---------------------------------------------------------------------------

def build_nc(cfg, plan, reps=1, ablate=()):
    """plan: from preprocess (gather call windows + per-block chunk cols).
    reps>1 repeats the computation (timing: the delta between an n-rep and
    1-rep program cancels the axon dispatch floor). ablate: subset of
    {"nogather","noonehot","noagg","nocoll"} for timing A/B experiments."""
    ablate = frozenset(ablate)
    N, D, CORES = cfg.N, cfg.D, cfg.CORES
    NB, NPAD = cfg.NB, cfg.NPAD
    C = plan["C"]
    calls = plan["calls"]
    blk_chunks = plan["blk_chunks"]
    ohspans = plan["ohspans"]

    nc = bacc.Bacc("TRN2", target_bir_lowering=False, debug=False,
                   num_devices=CORES, num_swdge_queues=NQ)

    x_d = nc.dram_tensor("x", [N, D], BF16, kind="ExternalInput")
    xT_d = nc.dram_tensor("xT", [D, NPAD], BF16, kind="ExternalInput")
    idx1_d = nc.dram_tensor("idx1", [128, C * 8], I16, kind="ExternalInput")
    idx2_d = nc.dram_tensor("idx2", [128, C * 8], I16, kind="ExternalInput")
    ohd_d = nc.dram_tensor("ohd", [128, C * 128], FP8,
                           kind="ExternalInput")
    gslot_d = nc.dram_tensor("gslot", [128, NB], F32, kind="ExternalInput")
    w1c_d = nc.dram_tensor("W1c", [D, 2 * D], BF16, kind="ExternalInput")
    w2c_d = nc.dram_tensor("W2c", [D, 2 * D], BF16, kind="ExternalInput")
    b1_d = nc.dram_tensor("b1", [D, 1], F32, kind="ExternalInput")
    b2_d = nc.dram_tensor("b2", [D, 1], F32, kind="ExternalInput")
    iota_d = nc.dram_tensor("iota", [128, 128], BF16, kind="ExternalInput")
    id64_d = nc.dram_tensor("id64", [D, D], BF16, kind="ExternalInput")

    pool_d = nc.dram_tensor("pool", [128, D], F32, kind="ExternalOutput")

    h1_local = nc.dram_tensor("h1_local", [NPAD, D], BF16)
    h1_full = nc.dram_tensor("h1_full", [NPAD * CORES, D], BF16,
                             addr_space="Shared")

    with tile.TileContext(nc) as tc:
        with (
            tc.tile_pool(name="persist", bufs=1) as pp,
            tc.tile_pool(name="work", bufs=6) as wp,
            tc.tile_pool(name="h2p", bufs=8) as h2p,
            tc.tile_pool(name="gat", bufs=40) as gp,
            tc.tile_pool(name="ohp", bufs=9) as ohp,
            tc.tile_pool(name="agg", bufs=3, space="PSUM") as aggp,
            tc.tile_pool(name="ps", bufs=2, space="PSUM") as psp,
            tc.tile_pool(name="pool1", bufs=1, space="PSUM") as pool1,
        ):
            xT_s = pp.tile([D, NPAD], BF16, tag="xT")
            h1T_s = pp.tile([D, NPAD], BF16, tag="h1T")
            idx1_s = pp.tile([128, C * 8], I16, tag="idx1")
            idx2_s = pp.tile([128, C * 8], I16, tag="idx2")
            gslot_s = pp.tile([128, NB], F32, tag="gslot")
            w1c_s = pp.tile([D, 2 * D], BF16, tag="w1c")
            w2c_s = pp.tile([D, 2 * D], BF16, tag="w2c")
            b1_s = pp.tile([D, 1], F32, tag="b1")
            b2_s = pp.tile([D, 1], F32, tag="b2")
            iota_s = pp.tile([128, 128], BF16, tag="iota")
            id64_s = pp.tile([D, D], BF16, tag="id64")

            for t, d in [(xT_s, xT_d), (idx1_s, idx1_d), (idx2_s, idx2_d),
                         (gslot_s, gslot_d),
                         (w1c_s, w1c_d), (w2c_s, w2c_d), (b1_s, b1_d),
                         (b2_s, b2_d), (iota_s, iota_d), (id64_s, id64_d)]:
                nc.sync.dma_start(out=t[:], in_=d[:, :])

            pool_ps = pool1.tile([128, D], F32, tag="pool")
            qctr = [0]

            if ablate:
                gdummy = pp.tile([128, CALL_CHUNKS * 64], BF16, tag="gdum")
                for k8 in range(CALL_CHUNKS):
                    nc.sync.dma_start(out=gdummy[:, k8 * 64:(k8 + 1) * 64],
                                      in_=x_d[0:128, :])
                ohdummy = pp.tile([128, 128], FP8, tag="ohdum")
                nc.sync.dma_start(out=ohdummy[:], in_=ohd_d[:, 0:128])

            LAG = 2  # blocks of tail pipelining (PSUM agg bufs = LAG+1)

            def leaky(dst_ap, z_ps, b_s):
                if USE_LRELU:
                    nc.scalar.activation(out=dst_ap, in_=z_ps[:],
                                         func=ACTF.Lrelu, bias=b_s[:, 0:1],
                                         alpha=cfg.LEAKY)
                else:
                    zb = wp.tile([D, 128], BF16, tag="zb")
                    nc.scalar.activation(out=zb[:], in_=z_ps[:],
                                         func=ACTF.Identity,
                                         bias=b_s[:, 0:1])
                    t = wp.tile([D, 128], BF16, tag="zt")
                    nc.vector.tensor_scalar_mul(out=t[:], in0=zb[:],
                                                scalar1=cfg.LEAKY)
                    nc.vector.tensor_tensor(out=dst_ap, in0=zb[:], in1=t[:],
                                            op=ALU.max)

            def layer(idx_s, table_d, wc_s, b_s, xfm_s, tail1_dst,
                      tail2_fn):
                # strided views: row stride 4 rows (512B), base offset r rows
                tabv = table_d[:, :].rearrange("(a b) f -> a (b f)", b=NRES)
                gtiles = []          # (c0, kk, tile) in call order
                pending1 = []        # blocks awaiting stage-1 tail
                pending2 = []        # blocks awaiting stage-2 tail

                def tail1(b, agg_ps):
                    # stage 1, LAG1 blocks late: PSUM->SBUF copy (ACT), the
                    # two accumulating z matmuls (PE), leaky (ACT). By now
                    # the agg accumulation finished LAG1 blocks ago, so the
                    # ACT copy at the queue head never waits on PE.
                    agg_sb = wp.tile([D, 128], BF16, tag="aggsb")
                    nc.scalar.activation(out=agg_sb[:], in_=agg_ps[:],
                                         func=ACTF.Copy)
                    z_ps = psp.tile([D, 128], F32, tag="z")
                    nc.tensor.matmul(out=z_ps[:], lhsT=wc_s[:, 0:D],
                                     rhs=agg_sb[:], start=True, stop=False)
                    nc.tensor.matmul(out=z_ps[:], lhsT=wc_s[:, D:2 * D],
                                     rhs=xfm_s[:, b * 128:(b + 1) * 128],
                                     start=False, stop=True)
                    leaky(tail1_dst(b), z_ps, b_s)
                    pending2.append(b)

                def flush_calls(upto_chunk):
                    # issue gather calls covering chunks < upto_chunk
                    while len(gtiles) < len(calls):
                        r, c0, kk = calls[len(gtiles)]
                        if c0 >= upto_chunk:
                            break
                        if "nogather" in ablate:
                            gtiles.append((c0, kk, gdummy))
                            continue
                        g = gp.tile([128, kk * 64], BF16, tag="g")
                        gv = g[:].rearrange("p (c f) -> p c f", c=kk)
                        _dma_gather_any(
                            nc, gv, tabv[:, r * 64:(r + 1) * 64],
                            idx_s[:, c0 * 8:(c0 + kk) * 8],
                            kk * 128, 64, NRES * 64, qctr[0] % NQ)
                        qctr[0] += 1
                        gtiles.append((c0, kk, g))

                call_of = {}
                for ci, (r, c0, kk) in enumerate(calls):
                    for cc in range(c0, c0 + kk):
                        call_of[cc] = ci

                ohtiles = []         # (c0, c1, tile) per (sb, r) span

                def flush_oh(upto_chunk):
                    while len(ohtiles) < len(ohspans):
                        c0, c1 = ohspans[len(ohtiles)]
                        if c0 >= upto_chunk:
                            break
                        if "noonehot" in ablate:
                            ohtiles.append((c0, c1, None))
                            continue
                        t = ohp.tile([128, (c1 - c0) * 128], FP8, tag="oh")
                        nc.sync.dma_start(
                            out=t[:], in_=ohd_d[:, c0 * 128:c1 * 128])
                        ohtiles.append((c0, c1, t))

                span_of = {}
                for si, (c0, c1) in enumerate(ohspans):
                    for cc in range(c0, c1):
                        span_of[cc] = si

                for b in range(NB):
                    cols = blk_chunks[b]
                    need = max(cols) + 1 if cols else 0
                    flush_calls(need)
                    flush_oh(need + 25)
                    agg_ps = aggp.tile([D, 128], F32, tag="agg")
                    total = len(cols)
                    work = []
                    for cc in cols:
                        ci = call_of[cc]
                        c0, kk, g = gtiles[ci]
                        col = cc - c0
                        o0, o1, ot = ohtiles[span_of[cc]]
                        if ot is None:
                            work.append((g, col, ohdummy, 0))
                        else:
                            work.append((g, col, ot, cc - o0))
                    if "noagg" in ablate:
                        nc.tensor.matmul(
                            out=agg_ps[:], lhsT=gdummy[:, 0:64],
                            rhs=ohdummy[:], start=True, stop=True)
                        work = []
                    done = 0
                    for (g, col, ot, ocol) in work:
                        nc.tensor.matmul(
                            out=agg_ps[:],
                            lhsT=g[:, col * 64:(col + 1) * 64],
                            rhs=ot[:, ocol * 128:(ocol + 1) * 128],
                            start=(done == 0),
                            stop=(done == total - 1))
                        done += 1
                    pending1.append((b, agg_ps))
                    if len(pending1) > LAG:
                        tail1(*pending1.pop(0))
                    if len(pending2) > LAG:
                        tail2_fn(pending2.pop(0))
                while pending1:
                    tail1(*pending1.pop(0))
                while pending2:
                    tail2_fn(pending2.pop(0))

            def l1_dst(b):
                return h1T_s[:, b * 128:(b + 1) * 128]

            def l1_tail2(b):
                # stage 2, 2*LAG blocks late: by now leaky(b) is long done,
                # so PE never stalls at the transpose. h1 rows go to DRAM
                # straight from PSUM (no compute-engine copy).
                t_ps = psp.tile([128, D], BF16, tag="tp")
                nc.tensor.transpose(out=t_ps[:],
                                    in_=h1T_s[:, b * 128:(b + 1) * 128],
                                    identity=id64_s[:])
                h1nm = wp.tile([128, D], BF16, tag="h1nm")
                nc.scalar.activation(out=h1nm[:], in_=t_ps[:],
                                     func=ACTF.Copy)
                nc.sync.dma_start(out=h1_local[b * 128:(b + 1) * 128, :],
                                  in_=h1nm[:])

            for _rep in range(reps):
                layer(idx1_s, x_d, w1c_s, b1_s, xT_s, l1_dst, l1_tail2)

                if "nocoll" not in ablate:
                    rows_k = NPAD // KCH
                    for k in range(KCH):
                        nc.gpsimd.collective_compute(
                            "AllGather",
                            ALU.bypass,
                            replica_groups=[list(range(CORES))],
                            ins=[h1_local[k * rows_k:(k + 1) * rows_k, :]],
                            outs=[h1_full[k * CORES * rows_k:
                                          (k + 1) * CORES * rows_k, :]],
                        )

                h2fm_tiles = {}

                def l2_dst(b):
                    t = h2p.tile([D, 128], BF16, tag="h2fm")
                    h2fm_tiles[b] = t
                    return t[:]

                def l2_tail2(b):
                    t_ps = psp.tile([128, D], BF16, tag="tp")
                    nc.tensor.transpose(out=t_ps[:],
                                        in_=h2fm_tiles.pop(b)[:],
                                        identity=id64_s[:])
                    h2nm = wp.tile([128, D], BF16, tag="h2nm")
                    nc.scalar.activation(out=h2nm[:], in_=t_ps[:],
                                         func=ACTF.Copy)
                    ph = wp.tile([128, 128], BF16, tag="ph")
                    nc.vector.tensor_scalar(
                        out=ph[:], in0=iota_s[:],
                        scalar1=gslot_s[:, b:b + 1], scalar2=None,
                        op0=ALU.is_equal)
                    nc.tensor.matmul(out=pool_ps[:], lhsT=ph[:],
                                     rhs=h2nm[:],
                                     start=(b == 0), stop=(b == NB - 1))

                layer(idx2_s, h1_full, w2c_s, b2_s, h1T_s, l2_dst, l2_tail2)

            pool_s = wp.tile([128, D], F32, tag="pools")
            nc.scalar.activation(out=pool_s[:], in_=pool_ps[:],
                                 func=ACTF.Copy)
            nc.sync.dma_start(out=pool_d[:, :], in_=pool_s[:])

    nc.compile()
    return nc


# ---------------------------------------------------------------------------
# Entry point
# ---------------------------------------------------------------------------

_CACHE = {}


def _common_inputs(cfg, W1_root, W1_rel, W2_root, W2_rel, b1, b2):
    D = cfg.D
    import jax.numpy as jnp

    def bf(a):
        return np.asarray(jnp.asarray(np.asarray(a, np.float32),
                                      dtype=jnp.bfloat16))

    return {
        "W1c": bf(np.concatenate([W1_rel, W1_root], axis=1)),
        "W2c": bf(np.concatenate([W2_rel, W2_root], axis=1)),
        "b1": np.ascontiguousarray(
            np.asarray(b1, np.float32).reshape(D, 1)),
        "b2": np.ascontiguousarray(
            np.asarray(b2, np.float32).reshape(D, 1)),
        "iota": bf(np.broadcast_to(np.arange(128, dtype=np.float32),
                                   (128, 128)).copy()),
        "id64": bf(np.eye(D, dtype=np.float32)),
    }


def _plan_key(plan):
    return (tuple(plan["calls"]), tuple(plan["ohspans"]),
            tuple(tuple(c) for c in plan["blk_chunks"]), plan["C"])


def run(cfg, inputs, trace=False):
    x = np.asarray(inputs["x_embeddings"], dtype=np.float32)
    in_maps, plan, g_base = preprocess(
        cfg, x, inputs["edge_index"], inputs["weights"], inputs["batch"])
    common = _common_inputs(cfg, inputs["W1_root"], inputs["W1_rel"],
                            inputs["W2_root"], inputs["W2_rel"],
                            inputs["b1"], inputs["b2"])
    for m in in_maps:
        m.update(common)

    key = (cfg.N, cfg.E, _plan_key(plan))
    if key not in _CACHE:
        _CACHE[key] = build_nc(cfg, plan)
    nc = _CACHE[key]

    res = run_bass_kernel_spmd(nc, in_maps, core_ids=list(range(cfg.CORES)),
                               trace=trace)

    batch = np.asarray(inputs["batch"], dtype=np.int64)
    counts = np.bincount(batch, minlength=cfg.G).astype(np.float32)
    pooled = np.zeros((cfg.G + 128, cfg.D), dtype=np.float32)
    for c in range(cfg.CORES):
        pooled[g_base[c]:g_base[c] + 128] += res.results[c]["pool"]
    pooled = pooled[:cfg.G] / np.maximum(counts, 1.0)[:, None]
    out = pooled @ np.asarray(inputs["Wl_root"], dtype=np.float32)
    out = out + np.asarray(inputs["bl"], dtype=np.float32)
    return out.astype(np.float32), res


def kernel(**inputs) -> np.ndarray:
    out, _ = run(REAL_CFG, inputs, trace=False)
    return out
